# revision 1
# baseline (speedup 1.0000x reference)
"""AttentionJKNET-GAT on 8 trn2 NeuronCores — fully on-device.

Nodes are permuted into degree-sorted slots dealt round-robin to 8 cores
(graph partitioned by dst). Each GAT layer gathers [h|ss|sd] rows for a
core's incoming edges with dma_gather (edge slots padded per 128-node round
so segment-softmax/segment-sum become fixed-stride vector reductions — no
scatter). Full gather tables are assembled with on-device AllGathers; the
per-node 3-token transformer runs sharded, feature-major. bf16 with f32
accumulation throughout.
"""
import contextlib
import numpy as np

P = 128
NCORES = 8
DIN = 128
D = 256
H = 4
L = 2
NEG = 0.2
HALF = 32768
EW = 384          # table row width (bf16) -> 768B, multiple of 256B
EPS = 1e-5
F = 448           # transformer nodes per tile
STAGES = 4        # debug: 1=tables 2=+gat1 3=+gat2 4=+transformer
GATOPS = 5        # debug: 1=gather 2=+escore 3=+aggregate 4=+transpose 5=+table2


# ===================================================================== host
def _table_ids(nrank, Sc):
    """rank -> table id (core-major): chunk j=rank//128, core=j%8, r=j//8."""
    j, p = nrank // P, nrank % P
    return (j % NCORES) * Sc + (j // NCORES) * P + p


def _preprocess(x, edge_index):
    import ml_dtypes
    N = x.shape[0]
    src = np.concatenate([edge_index[0].astype(np.int64), np.arange(N)])
    dst = np.concatenate([edge_index[1].astype(np.int64), np.arange(N)])
    E = src.size

    nchunk = -(-N // P)
    R = -(-nchunk // NCORES)
    Sc = R * P
    Npad = NCORES * Sc

    deg = np.bincount(dst, minlength=N)
    order1 = np.argsort(deg, kind="stable")
    rank1 = np.empty(N, np.int64)
    rank1[order1] = np.arange(N)
    tau1 = _table_ids(rank1, Sc)
    lo1 = np.bincount(dst[tau1[src] < HALF], minlength=N)
    order = np.lexsort((deg - lo1, lo1))
    rank = np.empty(N, np.int64)
    rank[order] = np.arange(N)
    tau = _table_ids(rank, Sc)

    srcT, dstT = tau[src], tau[dst]
    islo = srcT < HALF
    lo = np.bincount(dst[islo], minlength=N)
    hi = deg - lo

    lo_pad = np.zeros(R * NCORES * P, np.int64)
    hi_pad = np.zeros(R * NCORES * P, np.int64)
    lo_pad[:N] = lo[order]
    hi_pad[:N] = hi[order]
    KLs = lo_pad.reshape(R, NCORES * P).max(1)
    KHs = hi_pad.reshape(R, NCORES * P).max(1)
    Ks = KLs + KHs
    tokoff = np.zeros(R, np.int64)
    np.cumsum(Ks[:-1] * P, out=tokoff[1:])
    T_tok = int((Ks * P).sum())

    core_e = dstT // Sc
    local = dstT % Sc
    r_e = local // P
    p_e = local % P
    key = dstT * 2 + (~islo).astype(np.int64)
    ordk = np.argsort(key, kind="stable")
    counts = np.bincount(key, minlength=2 * Npad)
    starts = np.concatenate([[0], np.cumsum(counts)[:-1]])
    rk = np.empty(E, np.int64)
    rk[ordk] = np.arange(E) - starts[key[ordk]]
    col = np.where(islo, rk, KLs[r_e] + rk)
    tpos = tokoff[r_e] + col * P + p_e
    idxval = np.where(islo, srcT, srcT - HALF).astype(np.int16)

    idx_flat = np.zeros((NCORES, T_tok), np.int16)
    msk_flat = np.zeros((NCORES, T_tok), np.float32)
    idx_flat[core_e, tpos] = idxval
    msk_flat[core_e, tpos] = 1.0

    gidx, gmsk = [], []
    for c in range(NCORES):
        ia, ma = [], []
        for r in range(R):
            t0 = int(tokoff[r])
            nA, nB = int(P * KLs[r]), int(P * KHs[r])
            blk = idx_flat[c, t0:t0 + nA + nB]
            for seg in (blk[:nA], blk[nA:]):
                if seg.size:
                    ia.append(np.ascontiguousarray(seg.reshape(-1, 16).T))
            ma.append(msk_flat[c, t0:t0 + nA + nB].reshape(-1, P).T)
        gidx.append(np.ascontiguousarray(np.hstack(ia)))
        gmsk.append(np.ascontiguousarray(
            np.hstack(ma).astype(ml_dtypes.bfloat16)))
    gidx = np.stack(gidx)
    gmsk = np.stack(gmsk)

    allranks = np.arange(Npad).reshape(R, NCORES, P)
    node_of_rank = np.concatenate([order, np.zeros(Npad - N, np.int64)])
    xT = []
    for c in range(NCORES):
        ranks_c = allranks[:, c, :].reshape(-1)
        xc = x[node_of_rank[ranks_c]].astype(np.float32)
        xc[ranks_c >= N] = 0.0
        xT.append(np.ascontiguousarray(xc.T.astype(ml_dtypes.bfloat16)))
    xT = np.stack(xT)

    sched = tuple((int(a), int(b)) for a, b in zip(KLs, KHs))
    return dict(R=R, Sc=Sc, sched=sched, gidx=gidx, gmsk=gmsk,
                xT=xT, tau=tau, GI=gidx.shape[2], MC=gmsk.shape[2])


def _pack_weights(args):
    import ml_dtypes
    (gat1_W, gat1_b, gat1_asrc, gat1_adst, gat2_W, gat2_b, gat2_asrc,
     gat2_adst, cls_token, pos_emb, Wqkv, bqkv, Wo, bo, ln1_g, ln1_b,
     ln2_g, ln2_b, Wff1, bff1, Wff2, bff2, norm_g, norm_b) = args
    chunks, offs, cur = [], {}, 0

    def put(name, arr):
        nonlocal cur
        a = np.ascontiguousarray(arr, np.float32).reshape(-1)
        offs[name] = (cur, a.size)
        chunks.append(a)
        cur += a.size

    def aug(W, asrc, adst):
        K = W.shape[1]
        A = np.zeros((EW, K), np.float32)
        A[:D] = W
        A[D] = W.T @ asrc
        A[D + 1] = W.T @ adst
        return A.T                       # [K, EW]

    put("w1aug", aug(gat1_W, gat1_asrc, gat1_adst))
    put("w2aug", aug(gat2_W, gat2_asrc, gat2_adst))
    scale = 1.0 / np.sqrt(D // H)
    for l in range(L):
        wq = Wqkv[l].copy()
        wq[:D] *= scale
        put(f"wqkvT{l}", wq.T)
        put(f"woT{l}", Wo[l].T)
        put(f"wff1T{l}", Wff1[l].T)
        put(f"wff2T{l}", Wff2[l].T)
        bq = bqkv[l].copy()
        bq[:D] *= scale
        put(f"bqkv{l}", bq)
        put(f"bo{l}", bo[l])
        put(f"bff1{l}", bff1[l])
        put(f"bff2{l}", bff2[l])
        put(f"ln1g{l}", ln1_g[l]); put(f"ln1b{l}", ln1_b[l])
        put(f"ln2g{l}", ln2_g[l]); put(f"ln2b{l}", ln2_b[l])
    put("normg", norm_g); put("normb", norm_b)
    put("gatb1", gat1_b); put("gatb2", gat2_b)
    put("tok0", cls_token[0] + pos_emb[0])
    hm = np.zeros((2, P, H), np.float32)
    hmT = np.zeros((2, H, P), np.float32)
    for t in range(2):
        for hh in range(2):
            hm[t, hh * 64:(hh + 1) * 64, t * 2 + hh] = 1.0
            hmT[t, t * 2 + hh, hh * 64:(hh + 1) * 64] = 1.0
    put("hm0", hm[0]); put("hm1", hm[1])
    put("hmT0", hmT[0]); put("hmT1", hmT[1])
    put("pos1", pos_emb[1]); put("pos2", pos_emb[2])
    blob = np.concatenate(chunks)
    bw = -(-blob.size // (NCORES * P)) * (NCORES * P)
    blobp = np.zeros(bw, np.float32)
    blobp[: blob.size] = blob
    return blobp.astype(ml_dtypes.bfloat16), offs, bw


# ================================================================== program
def _build_program(R, Sc, sched, GI, MC, BW, WOFF):
    import concourse.bass as bass
    import concourse.tile as tile
    import concourse.mybir as mybir
    from concourse import bacc, masks

    f32 = mybir.dt.float32
    bf16 = mybir.dt.bfloat16
    i16 = mybir.dt.int16
    AF = mybir.ActivationFunctionType
    OP = mybir.AluOpType
    AX = mybir.AxisListType
    Stot = NCORES * Sc
    NT = Sc // F
    assert NT * F == Sc
    BC = BW // NCORES // P               # blob shard cols

    nc = bacc.Bacc("TRN2", target_bir_lowering=False, debug=False,
                   num_devices=NCORES)
    xT_d = nc.dram_tensor("xT", [DIN, Sc], bf16, kind="ExternalInput")
    gidx_d = nc.dram_tensor("gidx", [16, GI], i16, kind="ExternalInput")
    gmsk_d = nc.dram_tensor("gmsk", [P, MC], bf16, kind="ExternalInput")
    wsh_d = nc.dram_tensor("wshard", [P, BC], bf16, kind="ExternalInput")
    outT_d = nc.dram_tensor("outT", [D, Sc], bf16, kind="ExternalOutput")

    wshb = nc.dram_tensor("wshb", [P, BC], bf16)
    wfull = nc.dram_tensor("wfull", [NCORES * P, BC], bf16)
    t1sh = nc.dram_tensor("t1sh", [Sc, EW], bf16)
    t2sh = nc.dram_tensor("t2sh", [Sc, EW], bf16)
    table1 = nc.dram_tensor("table1", [Stot, EW], bf16)
    table2 = nc.dram_tensor("table2", [Stot, EW], bf16)
    x1T_d = nc.dram_tensor("x1T", [D, Sc], bf16)
    x2T_d = nc.dram_tensor("x2T", [D, Sc], bf16)

    def bc(ap, n):
        return bass.AP(ap.tensor, ap.offset, list(ap.ap) + [[0, n]])

    def view(ap, dims, extra_off=0):
        return bass.AP(ap.tensor, ap.offset + extra_off,
                       [list(ap.ap[0])] + dims)

    with tile.TileContext(nc) as tc:
        with contextlib.ExitStack() as ctx:
            cpool = ctx.enter_context(tc.tile_pool(name="consts", bufs=1))
            wpool = ctx.enter_context(tc.tile_pool(name="weights", bufs=1))

            ones_c = cpool.tile([P, 1], bf16)
            nc.vector.memset(ones_c[:], 1.0)
            ones_r = cpool.tile([1, P], bf16)
            nc.vector.memset(ones_r[:], 1.0)
            ident = cpool.tile([P, P], bf16)
            masks.make_identity(nc, ident[:])

            # ---- weight blob: bounce -> allgather -> tiles
            wtmp = cpool.tile([P, BC], bf16, tag="wtmp")
            nc.sync.dma_start(wtmp[:], wsh_d[:, :])
            nc.sync.dma_start(wshb[:, :], wtmp[:])
            nc.gpsimd.collective_compute(
                "AllGather", OP.bypass, replica_groups=[list(range(NCORES))],
                ins=[wshb.ap()], outs=[wfull.ap()])
            wf_t = wfull.ap().tensor

            def wmat(name, k, cols, rows=P):
                off, _ = WOFF[name]
                t = wpool.tile([rows, cols], bf16, tag=f"{name}_{k}")
                nc.sync.dma_start(
                    t[:], bass.AP(wf_t, off + k * rows * cols,
                                  [[cols, rows], [1, cols]]))
                return t

            def wcol(name, n=D):
                off, size = WOFF[name]
                assert size == n, (name, size, n)
                t = wpool.tile([P, n // P], f32, tag=name)
                nc.gpsimd.dma_start(
                    t[:], bass.AP(wf_t, off, [[1, P], [P, n // P]]))
                return t

            hmask = [wmat(f"hm{t}", 0, H) for t in range(2)]
            hmaskT = [wmat(f"hmT{t}", 0, P, rows=H) for t in range(2)]
            w1aug = wmat("w1aug", 0, EW)
            w2aug = [wmat("w2aug", k, EW) for k in range(2)]
            wqkvT = [[wmat(f"wqkvT{l}", k, 3 * D) for k in range(2)]
                     for l in range(L)]
            woT = [[wmat(f"woT{l}", k, D) for k in range(2)]
                   for l in range(L)]
            wff1T = [[wmat(f"wff1T{l}", k, 4 * D) for k in range(2)]
                     for l in range(L)]
            wff2T = [[wmat(f"wff2T{l}", k, D) for k in range(8)]
                     for l in range(L)]
            bqkv = [wcol(f"bqkv{l}", 3 * D) for l in range(L)]
            bo_ = [wcol(f"bo{l}") for l in range(L)]
            bff1 = [wcol(f"bff1{l}", 4 * D) for l in range(L)]
            bff2 = [wcol(f"bff2{l}") for l in range(L)]
            ln1g = [wcol(f"ln1g{l}") for l in range(L)]
            ln1b = [wcol(f"ln1b{l}") for l in range(L)]
            ln2g = [wcol(f"ln2g{l}") for l in range(L)]
            ln2b = [wcol(f"ln2b{l}") for l in range(L)]
            normg = wcol("normg")
            normb = wcol("normb")
            tok0 = wcol("tok0")
            pos1 = wcol("pos1")
            pos2 = wcol("pos2")

            # gat biases broadcast to [128, 256] f32
            gbias = []
            with tc.tile_pool(name="gb_ps", bufs=2, space="PSUM") as gbp:
                for name in ("gatb1", "gatb2"):
                    rrow = cpool.tile([1, D], bf16, tag=f"{name}_row")
                    nc.sync.dma_start(
                        rrow[:], bass.AP(wf_t, WOFF[name][0],
                                         [[D, 1], [1, D]]))
                    bcp = gbp.tile([P, D], f32, tag="gb")
                    nc.tensor.matmul(bcp[:], ones_r[:], rrow[:], start=True,
                                     stop=True)
                    gb = cpool.tile([P, D], f32, tag=f"{name}_bc")
                    nc.scalar.activation(gb[:], bcp[:], AF.Copy)
                    gbias.append(gb)

            gidx_sb = cpool.tile([P, GI], i16, tag="gidx")
            for k8 in range(8):
                nc.sync.dma_start(gidx_sb[16 * k8:16 * (k8 + 1), :],
                                  gidx_d[:, :])
            gmsk_sb = cpool.tile([P, MC], bf16, tag="gmsk")
            nc.sync.dma_start(gmsk_sb[:], gmsk_d[:, :])

            xsb = cpool.tile([DIN, Sc], bf16, tag="xsb")
            nc.sync.dma_start(xsb[:], xT_d[:, :])

            # ======== tables + GAT (phase-scoped pools) ========
            with contextlib.ExitStack() as gctx:
                tb_pool = gctx.enter_context(tc.tile_pool(name="tb", bufs=3))
                gpool = gctx.enter_context(tc.tile_pool(name="gat", bufs=2))
                spool = gctx.enter_context(tc.tile_pool(name="gsm", bufs=3))
                gps = gctx.enter_context(tc.tile_pool(name="gps", bufs=3,
                                                      space="PSUM"))

                for r in range(R):
                    ps = gps.tile([P, EW], f32, tag="g_ps")
                    nc.tensor.matmul(ps[:], xsb[:, r * P:(r + 1) * P],
                                     w1aug[:], start=True, stop=True)
                    ob = tb_pool.tile([P, EW], bf16, tag="tb_out")
                    nc.scalar.activation(ob[:], ps[:], AF.Copy)
                    nc.sync.dma_start(t1sh[r * P:(r + 1) * P, :], ob[:])
                nc.gpsimd.collective_compute(
                    "AllGather", OP.bypass,
                    replica_groups=[list(range(NCORES))],
                    ins=[t1sh.ap()], outs=[table1.ap()])

                def gat_layer(table, shard, gb_tile, nextw, next_shard,
                              xoutT_d):
                    acol = mcol = 0
                    for r in range(R):
                        KL, KH = sched[r]
                        K = KL + KH
                        g = gpool.tile([P, K * EW], bf16, tag="gath")
                        gap = g[:]
                        for c0 in range(0, KL, 8):
                            cw = min(8, KL - c0)
                            nc.gpsimd.dma_gather(
                                view(gap, [[EW, cw], [1, EW]], c0 * EW),
                                table[0:HALF, :],
                                gidx_sb[:, acol:acol + 8 * cw],
                                P * cw, P * cw, EW)
                            acol += 8 * cw
                        for c0 in range(0, KH, 8):
                            cw = min(8, KH - c0)
                            nc.gpsimd.dma_gather(
                                view(gap, [[EW, cw], [1, EW]],
                                     (KL + c0) * EW),
                                table[HALF:Stot, :],
                                gidx_sb[:, acol:acol + 8 * cw],
                                P * cw, P * cw, EW)
                            acol += 8 * cw
                        sd = spool.tile([P, 1], bf16, tag="sd")
                        nc.sync.dma_start(
                            sd[:], bass.AP(shard.ap().tensor,
                                           r * P * EW + (D + 1),
                                           [[EW, P], [1, 1]]))
                        if GATOPS < 2:
                            nc.sync.dma_start(
                                bass.AP(xoutT_d.ap().tensor, r * P,
                                        [[Sc, P], [1, P]]),
                                view(gap, [[1, P]]))
                            mcol += K
                            continue
                        e = spool.tile([P, K], f32, tag="e")
                        nc.vector.tensor_tensor(
                            e[:], view(gap, [[EW, K]], D), bc(sd[:], K),
                            OP.add)
                        nc.vector.scalar_tensor_tensor(
                            e[:], e[:], NEG, e[:], OP.mult, OP.max)
                        ex = spool.tile([P, K], f32, tag="ex")
                        nc.scalar.activation(ex[:], e[:], AF.Exp)
                        nc.vector.tensor_mul(ex[:], ex[:],
                                             gmsk_sb[:, mcol:mcol + K])
                        mcol += K
                        den = spool.tile([P, 1], f32, tag="den")
                        nc.vector.tensor_reduce(den[:], ex[:], AX.X, OP.add)
                        nc.vector.tensor_scalar_add(den[:], den[:], 1e-30)
                        rd = spool.tile([P, 1], f32, tag="rd")
                        nc.vector.reciprocal(rd[:], den[:])
                        if GATOPS < 3:
                            nc.sync.dma_start(
                                bass.AP(xoutT_d.ap().tensor, r * P,
                                        [[Sc, P], [1, 1]]), rd[:])
                            continue
                        w = gpool.tile([P, K * D], bf16, tag="wei")
                        nc.vector.tensor_tensor(
                            view(w[:], [[D, K], [1, D]]),
                            view(gap, [[EW, K], [1, D]]),
                            view(ex[:], [[1, K], [0, D]]), OP.mult)
                        num = spool.tile([P, D], f32, tag="num")
                        nc.vector.tensor_reduce(
                            num[:], view(w[:], [[1, D], [D, K]]), AX.X,
                            OP.add)
                        xo = spool.tile([P, D], bf16, tag="xo")
                        nc.vector.scalar_tensor_tensor(
                            xo[:], num[:], rd[:], gb_tile[:], OP.mult,
                            OP.add)
                        nc.scalar.activation(xo[:], xo[:], AF.Relu)
                        if GATOPS < 4:
                            nc.sync.dma_start(
                                bass.AP(xoutT_d.ap().tensor, r * P,
                                        [[Sc, P], [1, P]]), xo[:, :P])
                            continue
                        nps = gps.tile([P, EW], f32, tag="g_ps")
                        for f in range(2):
                            tp = gps.tile([P, P], bf16, tag="tr_ps")
                            nc.tensor.transpose(
                                tp[:], xo[:, f * P:(f + 1) * P], ident[:])
                            xt = spool.tile([P, P], bf16, tag="xt")
                            nc.scalar.activation(xt[:], tp[:], AF.Copy)
                            nc.sync.dma_start(
                                bass.AP(xoutT_d.ap().tensor,
                                        f * P * Sc + r * P,
                                        [[Sc, P], [1, P]]), xt[:])
                            if nextw is not None and GATOPS >= 5:
                                nc.tensor.matmul(nps[:], xt[:], nextw[f][:],
                                                 start=(f == 0),
                                                 stop=(f == 1))
                        if nextw is not None and GATOPS >= 5:
                            nb = tb_pool.tile([P, EW], bf16, tag="tb_out")
                            nc.scalar.activation(nb[:], nps[:], AF.Copy)
                            nc.sync.dma_start(
                                next_shard[r * P:(r + 1) * P, :], nb[:])

                if STAGES >= 2:
                    gat_layer(table1, t1sh, gbias[0], w2aug, t2sh, x1T_d)
                if STAGES >= 3:
                    nc.gpsimd.collective_compute(
                        "AllGather", OP.bypass,
                        replica_groups=[list(range(NCORES))],
                        ins=[t2sh.ap()], outs=[table2.ap()])
                    gat_layer(table2, t2sh, gbias[1], None, None, x2T_d)

            # ======== transformer ========
            with contextlib.ExitStack() as tctx:
                tpool = tctx.enter_context(tc.tile_pool(name="tf", bufs=2))
                t3 = tctx.enter_context(tc.tile_pool(name="tf3", bufs=2))
                psA = tctx.enter_context(tc.tile_pool(name="psA", bufs=4,
                                                      space="PSUM"))
                psB = tctx.enter_context(tc.tile_pool(name="psB", bufs=3,
                                                      space="PSUM"))

                def layer_norm(s, g_col, b_col):
                    mps = psB.tile([1, F], f32, tag="sm_ps")
                    sps = psB.tile([1, F], f32, tag="sm_ps")
                    for f in range(2):
                        sq = t3.tile([P, F], bf16, tag="ln_sq")
                        nc.scalar.activation(sq[:], s[f][:], AF.Square)
                        nc.tensor.matmul(mps[:], ones_c[:], s[f][:],
                                         start=(f == 0), stop=(f == 1))
                        nc.tensor.matmul(sps[:], ones_c[:], sq[:],
                                         start=(f == 0), stop=(f == 1))
                    m2 = t3.tile([1, F], f32, tag="ln_m2")
                    nc.vector.tensor_scalar_mul(m2[:], mps[:], 1.0 / D)
                    msq = t3.tile([1, F], f32, tag="ln_msq")
                    nc.vector.tensor_mul(msq[:], m2[:], m2[:])
                    va = t3.tile([1, F], f32, tag="ln_va")
                    nc.vector.scalar_tensor_tensor(
                        va[:], sps[:], 1.0 / D, msq[:], OP.mult, OP.subtract)
                    nc.vector.tensor_scalar_add(va[:], va[:], EPS)
                    vr = t3.tile([1, F], f32, tag="ln_vr")
                    nc.vector.reciprocal(vr[:], va[:])
                    rs = t3.tile([1, F], bf16, tag="ln_rs")
                    nc.scalar.activation(rs[:], vr[:], AF.Sqrt)
                    m2b = t3.tile([1, F], bf16, tag="ln_m2b")
                    nc.scalar.activation(m2b[:], m2[:], AF.Copy)
                    bmp = psB.tile([P, F], f32, tag="sm_ps")
                    nc.tensor.matmul(bmp[:], ones_r[:], m2b[:], start=True,
                                     stop=True)
                    brp = psB.tile([P, F], f32, tag="sm_ps")
                    nc.tensor.matmul(brp[:], ones_r[:], rs[:], start=True,
                                     stop=True)
                    for f in range(2):
                        xc = t3.tile([P, F], bf16, tag="ln_xc")
                        nc.vector.tensor_sub(xc[:], s[f][:], bmp[:])
                        nc.vector.tensor_mul(xc[:], xc[:], brp[:])
                        nc.vector.tensor_scalar(
                            s[f][:], xc[:], g_col[:, f:f + 1],
                            b_col[:, f:f + 1], OP.mult, OP.add)

                if STAGES >= 4:
                    with tc.For_i(0, NT, 1) as it:
                        off = it * F
                        s = [[tpool.tile([P, F], bf16, tag=f"s{t}_{f}", name=f"s{t}_{f}")
                              for f in range(2)] for t in range(3)]
                        for f in range(2):
                            nc.vector.tensor_copy(s[0][f][:],
                                                  bc(tok0[:, f:f + 1], F))
                            xin = t3.tile([P, F], bf16, tag="xin")
                            nc.sync.dma_start(
                                xin[:], x1T_d[f * P:(f + 1) * P,
                                              bass.ds(off, F)])
                            nc.vector.tensor_scalar_add(s[1][f][:], xin[:],
                                                        pos1[:, f:f + 1])
                            xin2 = t3.tile([P, F], bf16, tag="xin2")
                            nc.sync.dma_start(
                                xin2[:], x2T_d[f * P:(f + 1) * P,
                                               bass.ds(off, F)])
                            nc.vector.tensor_scalar_add(s[2][f][:], xin2[:],
                                                        pos2[:, f:f + 1])

                        for l in range(L):
                            qkv = [[None] * 6 for _ in range(3)]
                            for t in range(3):
                                for m in range(6):
                                    ps = psA.tile([P, F], f32, tag="mm_ps")
                                    for k in range(2):
                                        nc.tensor.matmul(
                                            ps[:],
                                            wqkvT[l][k][:, m * P:(m + 1) * P],
                                            s[t][k][:], start=(k == 0),
                                            stop=(k == 1))
                                    qt = tpool.tile([P, F], bf16,
                                                    tag=f"qkv{t}_{m}")
                                    nc.vector.tensor_scalar_add(
                                        qt[:], ps[:], bqkv[l][:, m:m + 1])
                                    qkv[t][m] = qt
                            ee = [[None] * 3 for _ in range(3)]
                            for i_ in range(3):
                                for j in range(3):
                                    lps = psB.tile([H, F], f32, tag="sm_ps")
                                    for f in range(2):
                                        pr = t3.tile([P, F], bf16, tag="qk")
                                        nc.vector.tensor_mul(
                                            pr[:], qkv[i_][f][:],
                                            qkv[j][2 + f][:])
                                        nc.tensor.matmul(lps[:], hmask[f][:],
                                                         pr[:], start=(f == 0),
                                                         stop=(f == 1))
                                    eij = t3.tile([H, F], bf16, tag=f"e{i_}{j}")
                                    nc.scalar.activation(eij[:], lps[:], AF.Exp)
                                    ee[i_][j] = eij
                            o = [[None] * 2 for _ in range(3)]
                            for i_ in range(3):
                                ssum = t3.tile([H, F], f32, tag="ssum")
                                nc.vector.tensor_add(ssum[:], ee[i_][0][:],
                                                     ee[i_][1][:])
                                nc.vector.tensor_add(ssum[:], ssum[:],
                                                     ee[i_][2][:])
                                recf = t3.tile([H, F], f32, tag="recf")
                                nc.vector.reciprocal(recf[:], ssum[:])
                                rec = t3.tile([H, F], bf16, tag="rec")
                                nc.vector.tensor_copy(rec[:], recf[:])
                                for j in range(3):
                                    al = t3.tile([H, F], bf16, tag="alpha")
                                    nc.vector.tensor_mul(al[:], ee[i_][j][:],
                                                         rec[:])
                                    for f in range(2):
                                        bps = psB.tile([P, F], f32, tag="sm_ps")
                                        nc.tensor.matmul(bps[:], hmaskT[f][:],
                                                         al[:], start=True,
                                                         stop=True)
                                        if j == 0:
                                            ot = t3.tile([P, F], bf16,
                                                         tag=f"o{i_}_{f}")
                                            nc.vector.tensor_mul(
                                                ot[:], bps[:], qkv[j][4 + f][:])
                                            o[i_][f] = ot
                                        else:
                                            tmp = t3.tile([P, F], bf16,
                                                          tag="otmp")
                                            nc.vector.tensor_mul(
                                                tmp[:], bps[:],
                                                qkv[j][4 + f][:])
                                            nc.vector.tensor_add(
                                                o[i_][f][:], o[i_][f][:],
                                                tmp[:])
                            for t in range(3):
                                ob = [t3.tile([P, F], bf16, tag=f"ob{f}", name=f"ob{f}")
                                      for f in range(2)]
                                for f in range(2):
                                    nc.scalar.activation(ob[f][:], o[t][f][:],
                                                         AF.Copy)
                                for fo in range(2):
                                    ps = psA.tile([P, F], f32, tag="mm_ps")
                                    for k in range(2):
                                        nc.tensor.matmul(
                                            ps[:],
                                            woT[l][k][:, fo * P:(fo + 1) * P],
                                            ob[k][:], start=(k == 0),
                                            stop=(k == 1))
                                    nc.vector.scalar_tensor_tensor(
                                        s[t][fo][:], ps[:], bo_[l][:, fo:fo + 1],
                                        s[t][fo][:], OP.add, OP.add)
                                layer_norm(s[t], ln1g[l], ln1b[l])
                            for t in range(3):
                                hh = []
                                for m in range(8):
                                    ps = psA.tile([P, F], f32, tag="mm_ps")
                                    for k in range(2):
                                        nc.tensor.matmul(
                                            ps[:],
                                            wff1T[l][k][:, m * P:(m + 1) * P],
                                            s[t][k][:], start=(k == 0),
                                            stop=(k == 1))
                                    ht = tpool.tile([P, F], bf16, tag=f"ffh{m}")
                                    nc.scalar.activation(
                                        ht[:], ps[:], AF.Relu,
                                        bias=bff1[l][:, m:m + 1])
                                    hh.append(ht)
                                for fo in range(2):
                                    ps = psA.tile([P, F], f32, tag="mm_ps")
                                    for k in range(8):
                                        nc.tensor.matmul(
                                            ps[:],
                                            wff2T[l][k][:, fo * P:(fo + 1) * P],
                                            hh[k][:], start=(k == 0),
                                            stop=(k == 7))
                                    nc.vector.scalar_tensor_tensor(
                                        s[t][fo][:], ps[:],
                                        bff2[l][:, fo:fo + 1], s[t][fo][:],
                                        OP.add, OP.add)
                                layer_norm(s[t], ln2g[l], ln2b[l])

                        layer_norm(s[0], normg, normb)
                        for f in range(2):
                            nc.sync.dma_start(
                                outT_d[f * P:(f + 1) * P, bass.ds(off, F)],
                                s[0][f][:])
    nc.finalize()
    return nc



def _run_spmd_fast(nc, in_maps):
    """run_bass_via_pjrt multi-core path, but donated output zero-buffers are
    created on-device (sharded jnp.zeros) instead of shipping ~27MB of host
    zeros through the axon tunnel. Kernel writes every output element."""
    import jax
    import numpy as _np
    from jax.sharding import Mesh, PartitionSpec, NamedSharding
    from jax.experimental.shard_map import shard_map
    from concourse import bass2jax as _b2j
    from concourse import mybir as _mybir

    _b2j.install_neuronx_cc_hook()
    assert nc.dbg_addr is None
    pid_name = (nc.partition_id_tensor.name
                if nc.partition_id_tensor is not None else None)
    in_names, out_names, out_avals = [], [], []
    zero_specs = []
    for alloc in nc.m.functions[0].allocations:
        if not isinstance(alloc, _mybir.MemoryLocationSet):
            continue
        name = alloc.memorylocations[0].name
        if alloc.kind == "ExternalInput":
            if name != pid_name:
                in_names.append(name)
        elif alloc.kind == "ExternalOutput":
            out_names.append(name)
            shape = tuple(alloc.tensor_shape)
            dtype = _mybir.dt.np(alloc.dtype)
            out_avals.append(jax.core.ShapedArray(shape, dtype))
            zero_specs.append((shape, dtype))
    n_params = len(in_names)
    n_outs = len(out_avals)
    all_in_names = tuple(in_names) + tuple(out_names)
    if pid_name is not None:
        all_in_names = all_in_names + (pid_name,)
    donate = tuple(range(n_params, n_params + n_outs))
    n_cores = len(in_maps)

    def _body(*args):
        operands = list(args)
        if pid_name is not None:
            operands.append(_b2j.partition_id_tensor())
        outs = _b2j._bass_exec_p.bind(
            *operands, out_avals=tuple(out_avals), in_names=all_in_names,
            out_names=tuple(out_names), lowering_input_output_aliases=(),
            sim_require_finite=True, sim_require_nnan=True, nc=nc)
        return tuple(outs)

    devices = jax.devices()[:n_cores]
    mesh = Mesh(_np.asarray(devices), ("core",))
    spec = PartitionSpec("core")
    sharded = jax.jit(
        shard_map(_body, mesh=mesh, in_specs=(spec,) * (n_params + n_outs),
                  out_specs=(spec,) * n_outs, check_rep=False),
        donate_argnums=donate, keep_unused=True)
    concat_in = [
        _np.concatenate([_np.asarray(in_maps[c][nm]) for c in range(n_cores)],
                        axis=0) for nm in in_names]
    mkz = jax.jit(
        lambda: tuple(jax.numpy.zeros((n_cores * sh[0], *sh[1:]), dt)
                      for sh, dt in zero_specs),
        out_shardings=tuple(NamedSharding(mesh, spec) for _ in zero_specs))
    zeros = mkz()
    out_arrs = sharded(*concat_in, *zeros)
    return [
        {nm: _np.asarray(out_arrs[i]).reshape(n_cores, *out_avals[i].shape)[c]
         for i, nm in enumerate(out_names)}
        for c in range(n_cores)
    ]


_PROG_CACHE = {}


# =================================================================== kernel
def _warm_all():
    try:
        from concourse.isa import get_isa
        get_isa("TRN2")
    except Exception:
        pass
    try:
        import concourse.bass  # noqa
        import concourse.tile  # noqa
        from concourse import bacc  # noqa
        from concourse.bass_utils import run_bass_kernel_spmd  # noqa
        import jax
        jax.devices()
    except Exception:
        pass


import threading as _threading
_WARM = _threading.Thread(target=_warm_all, daemon=True)
_WARM.start()


def kernel(x, edge_index, gat1_W, gat1_b, gat1_asrc, gat1_adst,
           gat2_W, gat2_b, gat2_asrc, gat2_adst, cls_token, pos_emb,
           Wqkv, bqkv, Wo, bo, ln1_g, ln1_b, ln2_g, ln2_b,
           Wff1, bff1, Wff2, bff2, norm_g, norm_b):
    import os as _os
    import time as _time
    _dbg = bool(_os.environ.get("KERNEL_TIMING"))
    _t0 = _time.time()

    def _lap(msg):
        nonlocal _t0
        if _dbg:
            print(f"[kt] {msg}: {_time.time() - _t0:.2f}s", flush=True)
        _t0 = _time.time()


    x = np.asarray(x, np.float32)
    edge_index = np.asarray(edge_index)
    args = [np.asarray(a, np.float32) for a in
            (gat1_W, gat1_b, gat1_asrc, gat1_adst, gat2_W, gat2_b, gat2_asrc,
             gat2_adst, cls_token, pos_emb, Wqkv, bqkv, Wo, bo, ln1_g, ln1_b,
             ln2_g, ln2_b, Wff1, bff1, Wff2, bff2, norm_g, norm_b)]
    N = x.shape[0]

    pp = _preprocess(x, edge_index)
    blob, offs, BW = _pack_weights(args)
    _lap("preprocess+pack")

    key = (pp["R"], pp["sched"], pp["GI"], pp["MC"], BW)
    if key not in _PROG_CACHE:
        _PROG_CACHE[key] = _build_program(pp["R"], pp["Sc"], pp["sched"],
                                          pp["GI"], pp["MC"], BW, offs)
    nc = _PROG_CACHE[key]
    _lap("build")
    from concourse.bass_utils import run_bass_kernel_spmd
    _lap("imports")

    shard = BW // NCORES
    in_maps = [{
        "xT": pp["xT"][c],
        "gidx": pp["gidx"][c],
        "gmsk": pp["gmsk"][c],
        "wshard": blob[c * shard:(c + 1) * shard].reshape(P, -1),
    } for c in range(NCORES)]
    _WARM.join(timeout=120)
    _lap("warm join")
    try:
        res = _run_spmd_fast(nc, in_maps)
    except Exception:
        res = run_bass_kernel_spmd(nc, in_maps, list(range(NCORES))).results
    _lap("run")

    big = np.concatenate([np.asarray(r["outT"]) for r in res], axis=1)
    out = np.empty((N, D), np.float32)
    out[...] = big.T[pp["tau"][:N]]
    _lap("post")
    return out

